# revision 84
# baseline (speedup 1.0000x reference)
"""Trainium2 Bass kernel for nn_Capsule (capsule routing with dynamic routing).

reference: u = x @ W  (per-sample [512,256]@[256,512] -> [512, (32 o, 16 f)])
           b=0; 3x { c = softmax_o(b); v[o,f] = sum_i c[o,i] u[i,(o,f)];
                     v = squash(v); b[o,i] = sum_f v[o,f] u[i,(o,f)] }
           return v  [B, 32, 16]

Key algebraic restructuring (u is NEVER materialized):
  v_raw[o,f] = sum_i c[o,i] u[i,(o,f)]  =  diag-extract[ (c @ x) @ W ]
      yT[h,(s,o)] = x-chunk stationary @ cT moving   (PE bf16, 32-wide)
      vfull = y @ W  (PE fp32r: yT stationary, W natural moving)
      v_raw = mask * vfull, then per-sample partition-sum via indicator matmul
  b[o,i] = sum_f v[o,f] u[i,(o,f)] = sum_h z[o,h] x[i,h]
      zT[h,(s,o)] = WT-chunk stationary @ VmatT moving (PE fp32r, dst-0)
      bT[i,(s,o)] = xT-chunk stationary @ zT moving    (PE bf16, 32-wide)
  softmax over o directly on bT [i-partition, o-free]; cT feeds next yT.

Dtype split: fp32r (fp32 bits, PE fast path, dst partition 0 only) for
vfull / vr / flipped-z; bf16 for x / xT / cT / zT / exp(b) and the flipped
y and b phases. x is cast to bf16 on the HOST (halves HBM traffic).

The two half-batches are interleaved per routing iteration (A=aggregation
phase, B=agreement phase: emit A0 A1 B0 B1) so each half-batch's serial
squash/softmax latency hides under the other's PE work, and the Activation
engine runs Sqrt x2 then Exp x2 back-to-back (2 table loads per iteration).
"""

import numpy as np
import ml_dtypes

import concourse.bass as bass
import concourse.tile as tile
from concourse import mybir
from concourse.bass_utils import run_bass_kernel_spmd

F32 = mybir.dt.float32
R32 = mybir.dt.float32r
BF16 = mybir.dt.bfloat16
I32 = mybir.dt.int32
AF = mybir.ActivationFunctionType
AX = mybir.AxisListType

B, I, H = 128, 512, 256
O, F = 32, 16
OF = O * F  # 512
NCORES = 8
S = B // NCORES      # 16 samples per core
NHB = 2              # half-batches per core
NPK = 2              # packs per half-batch
PK = 4               # samples per pack
NITER = 3
P = 128

# fp32 constant blob layout (one DMA, per-partition element offsets)
CW = 0                  # W  [h%128, (hc 2, of 512)]
CWT = CW + 2 * OF       # WT [of%128, (m 4, h 256)]
CID = CWT + 4 * H       # identity [128, 128]
CMC = CID + P           # diag mask [128, 512]
CS4 = CMC + OF          # sample-sum indicator [128, 4]
CBM = CS4 + PK          # Vmat block masks [128, (m 4, j 32)]
CSTN = CBM + 4 * O

# bf16 constant blob
CBID = 0                # identity [128, 128] bf16
CBC0 = CBID + P         # uniform 1/32 [128, 32] bf16
CBN = CBC0 + O


def ap(t, dims, off=0):
    """AP over tile/handle `t`: keep partition dim, explicit free dims."""
    a = t if isinstance(t, bass.AP) else t[:]
    return bass.AP(tensor=a.tensor, offset=a.offset + off,
                   ap=[list(a.ap[0])] + [list(d) for d in dims])


def fview(a):
    """Alias a float32r AP as plain fp32 (same bytes) for transposes/DVE."""
    t = a.tensor
    if t.dtype != R32:
        return a
    t2 = bass.SBTensorHandle(name=t.name, shape=t.shape, dtype=F32,
                             base_partition=t.base_partition,
                             manual_sbuf_range=t.manual_sbuf_range,
                             manual_base_name=t.manual_base_name)
    return bass.AP(tensor=t2, offset=a.offset,
                   ap=[list(d) for d in a.ap])


def aview(a, dt):
    """Alias an AP as another same-width dtype (bit reinterpret, READ only:
    tile write tracking does not see writes through an aliased handle)."""
    t = a.tensor
    t2 = bass.SBTensorHandle(name=t.name, shape=t.shape, dtype=dt,
                             base_partition=t.base_partition,
                             manual_sbuf_range=t.manual_sbuf_range,
                             manual_base_name=t.manual_base_name)
    return bass.AP(tensor=t2, offset=a.offset,
                   ap=[list(d) for d in a.ap])


def dram_ap(handle, dims, off=0):
    """AP over DRAM handle with fully explicit dims (first = partition)."""
    a = handle[:]
    return bass.AP(tensor=a.tensor, offset=a.offset + off,
                   ap=[list(d) for d in dims])


def build_program(split_waits=True, loop_n=None):
    """loop_n: wrap the whole body in a hardware For_i loop (timing runs)."""
    import contextlib

    nc = bass.Bass("TRN2", target_bir_lowering=False)

    x_d = nc.dram_tensor("x", [S, I, H], BF16, kind="ExternalInput")
    cst_d = nc.dram_tensor("cst", [P, CSTN], R32, kind="ExternalInput")
    cstb_d = nc.dram_tensor("cstb", [P, CBN], BF16, kind="ExternalInput")
    out_d = nc.dram_tensor("out", [S, OF], F32, kind="ExternalOutput")

    with tile.TileContext(nc) as tc:
        with (
            tc.tile_pool(name="consts", bufs=1) as consts,
            tc.tile_pool(name="xpool", bufs=4) as xpool,
            tc.tile_pool(name="xtpool", bufs=4) as xtpool,
            tc.tile_pool(name="work", bufs=3) as work,
            tc.tile_pool(name="sm", bufs=8) as sm,
            tc.tile_pool(name="ps2k", bufs=5, space="PSUM") as ps2k,
            tc.tile_pool(name="ps_xt", bufs=2, space="PSUM") as ps_xt,
            tc.tile_pool(name="ps_anch", bufs=1, space="PSUM") as ps_anch,
            tc.For_i(0, loop_n, 1) if loop_n else contextlib.nullcontext(),
        ):
            # ---- constants ----
            cstb = consts.tile([P, CBN], BF16)
            nc.sync.dma_start(out=cstb[:], in_=cstb_d[:])
            idb_sb = cstb[:, CBID:CBID + P]      # identity bf16
            c0b_sb = cstb[:, CBC0:CBC0 + O]      # uniform 1/32 bf16

            cst = consts.tile([P, CSTN], R32)
            w_sb = cst[:, CW:CW + 2 * OF]        # [h%128, (hc, of)] R32
            wt_sb = cst[:, CWT:CWT + 4 * H]      # [of%128, (m, h)] R32
            idr_sb = cst[:, CID:CID + P]         # identity (R32 transposes)
            id_sb = fview(idr_sb)                # identity (fp32 transposes)
            mc_sb = fview(cst[:, CMC:CMC + OF])  # diag mask (p%32 == o)
            s4_sb = cst[:, CS4:CS4 + PK]         # s4[p,s] = (p//32 == s) R32
            bm_sb = fview(cst[:, CBM:CBM + 4 * O])  # bm[p,(m,j)]=(j==8m+p//16)

            # PE sync anchors: every datapath instruction carries at most ONE
            # sem wait (walrus).  A 1x1 transpose reading a byte of a dirty
            # foreign-engine tensor makes PE "observe" that engine's clock so
            # later PE instructions need no cross-engine waits.
            anch = ps_anch.tile([P, F], F32)
            # bf16 alias of the same PSUM bytes (even cols: 4B alignment)
            anchb = anch[:].tensor.bitcast(BF16)[:]
            dirty = {}
            acol = [0]
            pending = []

            def mark(key, apv):
                dirty[key] = apv

            def pe_sync(*keys):
                pending.clear()
                for k in keys:
                    if k not in dirty:
                        continue
                    d = dirty.pop(k)
                    dd = fview(d[:1, :1])
                    if dd.tensor.dtype == BF16:
                        a = nc.tensor.transpose(
                            anchb[:1, 2 * acol[0]:2 * acol[0] + 1], dd,
                            idb_sb[:1, :1])
                    else:
                        a = nc.tensor.transpose(
                            anch[:1, acol[0]:acol[0] + 1], dd,
                            id_sb[:1, :1])
                    pending.append(a.ins)
                    acol[0] = (acol[0] + 1) % F

            def _chain(b):
                for a in pending:
                    bass._add_dep_helper(b.ins, a, sync=False,
                                         reason="pe-anchor order")
                return b

            def T(out, in_, ident):
                return _chain(nc.tensor.transpose(out, in_, ident))

            def MM(out, lhsT, rhs, **kw):
                return _chain(nc.tensor.matmul(out, lhsT, rhs, **kw))

            def dep(b, a):
                if a is not None:
                    bass._add_dep_helper(b.ins, a, sync=False,
                                         reason="engine-anchor order")
                return b

            mark("cstb", cstb)

            dscr = sm.tile([PK, PK], F32, tag="dscr")
            nc.vector.memset(dscr[:], 0.0)

            # ---- x loads (bf16, natural [i, h]) ----
            x_sb = {}   # (hb, pk) -> flat [128, (s, ic, h)] = [128, 4096]
            xt_sb = {}  # (hb, pk) -> flat [128, (s, hc, i)] = [128, 4096]
            for hb in range(NHB):
                for pk in range(NPK):
                    samp0 = hb * 8 + pk * 4
                    xs = xpool.tile([P, PK * 4 * H], BF16, tag="x")
                    # per-sample chunks: transposes/y start after 1/4 tile
                    for s in range(PK):
                        nc.sync.dma_start(
                            out=ap(xs, [[H, 4], [1, H]], off=s * 4 * H),
                            in_=dram_ap(x_d, [[H, P], [P * H, 4], [1, H]],
                                        off=(samp0 + s) * I * H),
                        )
                        mark("x%d%d%d" % (hb, pk, s), xs[:, s * 4 * H:
                                                        s * 4 * H + 1])
                    x_sb[(hb, pk)] = xs
                if hb == 0:
                    # consts DMA between the two half-batches' x loads:
                    # cst is first needed at vfull(hb0,t0).
                    nc.sync.dma_start(out=cst[:], in_=cst_d[:])
                    mark("cst", cst)
            # one-time: let DVE observe the const DMA (mc/bm reads)
            dcst_a = nc.vector.tensor_copy(dscr[:1, :1],
                                           fview(cst[:1, :1])).ins

            def build_xt(hb):
                """xT via PE transposes; copies split DVE/Act/Pool."""
                for pk in range(NPK):
                    xs = x_sb[(hb, pk)]
                    xt = xtpool.tile([P, PK * 2 * I], BF16, tag="xt")
                    for s in range(PK):
                        pe_sync("cstb", "x%d%d%d" % (hb, pk, s),
                                "act", "dve", "gp")
                        pxt = ps_xt.tile([P, 2 * I], BF16, tag="pxt")
                        for hc in range(2):
                            for ic in range(4):
                                T(
                                    pxt[:, hc * I + ic * P:
                                        hc * I + (ic + 1) * P],
                                    xs[:, s * 1024 + ic * H + hc * P:
                                           s * 1024 + ic * H + (hc + 1) * P],
                                    idb_sb,
                                )
                        # Act is idle until the first Exp (~35us): give it
                        # most xT copies, freeing DVE for the t0 chain.
                        dst = xt[:, s * 1024: (s + 1) * 1024]
                        if s % 2 == 0:
                            nc.scalar.activation(dst, pxt[:], AF.Copy)
                            mark("act", dst)
                        else:
                            nc.vector.tensor_copy(dst, pxt[:])
                            mark("dve", dst)
                    xt_sb[(hb, pk)] = xt

            cT = {0: None, 1: None}  # per-hb bf16 [128 i%128, (ic, pk, s, o)]
            vstate = {}              # per-hb carry between A and B phases

            def phase_A(hb, t):
                """y/vfull/vr/squash (through vsq): aggregation phase."""
                ve = nc.vector if hb == 0 else nc.gpsimd
                vkey = "dve" if hb == 0 else "gp"
                pvr_of = {}
                if t == 0:
                    # ---- t0 fast path: c uniform 1/32 -> every o gets the
                    # same y row, so v_raw = (xsum/32) @ W needs no diag
                    # extract: xsum via 1-wide matmuls, then a [h,4]-wide
                    # vfull lands v_raw per pack directly. ----
                    pe_sync("cstb", "dve", "act", "gp",
                            *["x%d%d%d" % (hb, pk_, s_)
                              for pk_ in range(NPK) for s_ in range(PK)])
                    ps_y0 = ps2k.tile([P, NPK * PK * 2], F32, tag="p2k")
                    for hc in range(2):
                        for pk in range(NPK):
                            for s in range(PK):
                                for ic in range(4):
                                    MM(
                                        ps_y0[:, hc * 8 + pk * PK + s:
                                              hc * 8 + pk * PK + s + 1],
                                        ap(x_sb[hb, pk], [[1, P]],
                                           off=s * 1024 + ic * H + hc * P),
                                        c0b_sb[:, :1],
                                        start=(ic == 0),
                                        stop=(ic == 3),
                                    )
                    yt0_sb = work.tile([P, NPK * PK * 2], R32, tag="yt0")
                    nc.vector.tensor_copy(yt0_sb[:], ps_y0[:])
                    mark("dve", yt0_sb)
                    pe_sync("dve", "cst")
                    for pk in range(NPK):
                        pvr = ps2k.tile([PK, OF], F32, tag="p2k")
                        for hc in range(2):
                            MM(
                                pvr[:],
                                ap(yt0_sb, [[1, PK]], off=hc * 8 + pk * PK),
                                ap(w_sb, [[1, OF]], off=hc * OF),
                                start=(hc == 0),
                                stop=(hc == 1),
                            )
                        pvr_of[pk] = pvr
                else:
                    # ---- yT[h,(hc,pk,s,o)] = x-stationary @ cT (bf16) ----
                    pe_sync("cstb", "dve", "act", "gp",
                            *["x%d%d%d" % (hb, pk_, s_)
                              for pk_ in range(NPK) for s_ in range(PK)])
                    ps_yt = ps2k.tile([P, NPK * H], F32, tag="p2k")
                    for hc in range(2):
                        for pk in range(NPK):
                            for s in range(PK):
                                for ic in range(4):
                                    MM(
                                        ps_yt[:, hc * 256 + pk * P + s * O:
                                              hc * 256 + pk * P + s * O + O],
                                        ap(x_sb[hb, pk], [[1, P]],
                                           off=s * 1024 + ic * H + hc * P),
                                        ap(cT[hb], [[1, O]],
                                           off=ic * 256 + pk * P + s * O),
                                        start=(ic == 0),
                                        stop=(ic == 3),
                                    )
                    yt_sb = work.tile([P, NPK * H], R32, tag="ytsb")
                    nc.vector.tensor_copy(yt_sb[:], ps_yt[:])
                    mark("dve", yt_sb)

                    # ---- vfull = y @ W (fp32r): [(pk,) 4s*32o', (o,f)] ----
                    pe_sync("dve", "cst")
                    msk_sb = work.tile([P, NPK * OF], R32, tag="bigsb")
                    for pk in range(NPK):
                        pvf = ps2k.tile([P, OF], F32, tag="p2k")
                        for hc in range(2):
                            MM(
                                pvf[:],
                                ap(yt_sb, [[1, P]], off=hc * 256 + pk * P),
                                ap(w_sb, [[1, OF]], off=hc * OF),
                                start=(hc == 0),
                                stop=(hc == 1),
                            )
                        # diag extract mask (release vf bank early per pk)
                        dep(nc.vector.tensor_mul(
                            ap(msk_sb, [[1, OF]], off=pk * OF),
                            pvf[:],
                            ap(mc_sb, [[1, OF]]),
                        ), dcst_a)
                        mark("dve", msk_sb[:, pk * OF:pk * OF + 1])

                # ---- per-sample partition sum (fp32r indicator), per pk ----
                pe_sync("dve")
                vr_sb = work.tile([PK, NPK * OF], F32, tag="vrsb")
                sq_sb = work.tile([PK, NPK * OF], F32, tag="sqsb")
                mag = sm.tile([PK, NPK * O], F32, tag="mag")
                red_ins = []
                for pk in range(NPK):
                    if t == 0:
                        pvr = pvr_of[pk]
                    else:
                        pvr = ps2k.tile([PK, OF], F32, tag="p2k")
                        MM(
                            pvr[:],
                            s4_sb,
                            msk_sb[:, pk * OF:(pk + 1) * OF],
                            start=True, stop=True,
                        )
                    # DVE may read at most one PSUM operand: land vr in
                    # SBUF once, square/scale from there (Pool-eligible)
                    nc.vector.tensor_copy(
                        ap(vr_sb, [[1, OF]], off=pk * OF), pvr[:])
                    ve.tensor_mul(
                        ap(sq_sb, [[1, OF]], off=pk * OF),
                        ap(vr_sb, [[1, OF]], off=pk * OF),
                        ap(vr_sb, [[1, OF]], off=pk * OF))
                red_ins.append(nc.vector.reduce_sum(
                    out=mag[:],
                    in_=ap(sq_sb, [[F, NPK * O], [1, F]]),
                    axis=AX.X,
                ).ins)

                # ---- squash: factor = sqrt(mag)/(1+mag), sqrt via rsqrt
                # bit-trick + 2 Newton steps (keeps the Act table on Exp).
                # hb0's scalar chain runs on DVE, hb1's on Pool so the two
                # half-batches don't contend.
                sh_t = sm.tile([PK, NPK * O], I32, tag="sh")
                # int shift is DVE-only (Pool ALU lacks it)
                sh_i = nc.vector.tensor_scalar(
                    out=sh_t[:], in0=aview(mag[:], I32), scalar1=1,
                    scalar2=None,
                    op0=mybir.AluOpType.arith_shift_right)
                for ri_ in red_ins:  # alias reads are not dep-tracked
                    bass._add_dep_helper(sh_i.ins, ri_, sync=True,
                                         reason="aliased mag read")
                r0_t = sm.tile([PK, NPK * O], I32, tag="s0")
                nc.vector.tensor_scalar(
                    out=r0_t[:], in0=sh_t[:], scalar1=-1,
                    scalar2=0x5F3759DF,
                    op0=mybir.AluOpType.mult, op1=mybir.AluOpType.add)
                r = aview(r0_t[:], F32)
                for _ in range(1):  # Newton: r = r*(1.5 - 0.5*m*r^2)
                    a = sm.tile([PK, NPK * O], F32, tag="nta")
                    ve.tensor_mul(a[:], mag[:], r)
                    h = sm.tile([PK, NPK * O], F32, tag="nth")
                    ve.tensor_mul(h[:], a[:], r)
                    w = sm.tile([PK, NPK * O], F32, tag="ntw")
                    ve.tensor_scalar(
                        out=w[:], in0=h[:], scalar1=-0.5, scalar2=1.5,
                        op0=mybir.AluOpType.mult, op1=mybir.AluOpType.add)
                    r2 = sm.tile([PK, NPK * O], F32, tag="ntr")
                    ve.tensor_mul(r2[:], r, w[:])
                    r = r2[:]
                s0 = sm.tile([PK, NPK * O], F32, tag="s0f")
                ve.tensor_mul(s0[:], mag[:], r)  # sqrt = m * rsqrt
                onep = sm.tile([PK, NPK * O], F32, tag="onep")
                ve.tensor_scalar_add(onep[:], mag[:], 1.0)
                rec = sm.tile([PK, NPK * O], F32, tag="rec")
                nc.vector.reciprocal(rec[:], onep[:])
                factor = sm.tile([PK, NPK * O], F32, tag="fac")
                ve.tensor_mul(factor[:], s0[:], rec[:])
                vsq = work.tile([PK, NPK * OF], F32, tag="vsq", bufs=4)
                for pk in range(NPK):
                    ve.tensor_mul(
                        ap(vsq, [[F, O], [1, F]], off=pk * OF),
                        ap(vr_sb, [[F, O], [1, F]], off=pk * OF),
                        ap(factor, [[1, O], [0, F]], off=pk * O),
                    )
                mark(vkey, vsq)
                vstate[hb] = vsq

            def phase_B(hb, t):
                """agreement phase: vT/vp2/zT/bT/softmax (skip on last iter)."""
                vsq = vstate[hb]

                if t == NITER - 1:
                    dq = nc.sync if hb == 0 else nc.scalar
                    dq.dma_start(
                        out=dram_ap(out_d, [[OF, PK], [PK * OF, NPK], [1, OF]],
                                    off=hb * 8 * OF),
                        in_=ap(vsq, [[OF, NPK], [1, OF]]),
                    )
                    return

                # ---- vT chunks: [(o8,f16)%128, (pk, m, s)] ----
                pe_sync("dve", "act")
                ps_vt = ps2k.tile([P, NPK * 4 * PK], F32, tag="p2k")
                for pk in range(NPK):
                    for m in range(4):
                        T(
                            ps_vt[:, (pk * 4 + m) * PK:(pk * 4 + m + 1) * PK],
                            vsq[:, pk * OF + m * P: pk * OF + (m + 1) * P],
                            id_sb[:PK, :PK],
                        )
                vt_sb = work.tile([P, NPK * 4 * PK], F32, tag="vtsb")
                nc.vector.tensor_copy(vt_sb[:], ps_vt[:])

                # ---- VmatT blocks: vp2[p,(m,pk,s,o)] = vtT * blockmask ----
                vp2_sb = work.tile([P, 4 * NPK * PK * O], R32, tag="vp")
                ve2 = nc.gpsimd
                for m in range(4):
                    dep(ve2.tensor_mul(
                        ap(vp2_sb, [[PK * O, NPK], [O, PK], [1, O]],
                           off=m * NPK * PK * O),
                        ap(vt_sb, [[4 * PK, NPK], [1, PK], [0, O]],
                           off=m * PK),
                        ap(bm_sb, [[0, NPK], [0, PK], [1, O]],
                           off=m * O),
                    ), dcst_a)
                mark("gp",
                     vp2_sb[:, 3 * NPK * PK * O:3 * NPK * PK * O + 1])

                # ---- zT = WT-chunk stationary @ VmatT (fp32r, dst 0):
                #      [h%128, (hc, pk, s, o)] ----
                pe_sync("dve", "act", "gp")
                ps_zt = ps2k.tile([P, NPK * H], F32, tag="p2k")
                for hc in range(2):
                    for m in range(4):
                        MM(
                            ps_zt[:, hc * NPK * P:(hc + 1) * NPK * P],
                            ap(wt_sb, [[1, P]], off=m * H + hc * P),
                            ap(vp2_sb, [[1, NPK * P]], off=m * NPK * P),
                            start=(m == 0),
                            stop=(m == 3),
                        )
                zt_sb = work.tile([P, NPK * H], BF16, tag="ztsb")
                nc.vector.tensor_copy(zt_sb[:], ps_zt[:])
                mark("dve", zt_sb)

                # ---- bT[i,(ic),(pk,s,o)] = xT-chunk stationary @ zT (bf16),
                #      two 2KB PSUM halves so exp(half0) overlaps half1 ----
                pe_sync("dve")
                ebt = work.tile([P, 4 * NPK * P], BF16, tag="ebt")
                for half in range(2):
                    ps_bt = ps2k.tile([P, NPK * P * 2], F32, tag="p2k")
                    for ic2 in range(2):
                        ic = half * 2 + ic2
                        for pk in range(NPK):
                            for s in range(PK):
                                for hc in range(2):
                                    MM(
                                        ps_bt[:, ic2 * 256 + pk * P + s * O:
                                              ic2 * 256 + pk * P + s * O + O],
                                        ap(xt_sb[hb, pk], [[1, P]],
                                           off=s * 1024 + hc * I + ic * P),
                                        ap(zt_sb, [[1, O]],
                                           off=hc * NPK * P + pk * P + s * O),
                                        start=(hc == 0),
                                        stop=(hc == 1),
                                    )
                    # ---- softmax exp (b in +-5: exp w/o max-sub) ----
                    nc.scalar.activation(
                        ebt[:, half * 512:(half + 1) * 512],
                        ps_bt[:], AF.Exp)
                mark("act", ebt)
                ssum = sm.tile([P, 4 * NPK * PK], F32, tag="ssum")
                nc.vector.reduce_sum(
                    out=ssum[:],
                    in_=ap(ebt, [[O, 4 * NPK * PK], [1, O]]),
                    axis=AX.X,
                )
                rsum = sm.tile([P, 4 * NPK * PK], F32, tag="rsum")
                nc.vector.reciprocal(rsum[:], ssum[:])
                cT[hb] = work.tile([P, 4 * NPK * P], BF16, tag="ct%d" % hb,
                                   name="ct_t")
                ve3 = nc.gpsimd
                ve3.tensor_mul(
                    ap(cT[hb], [[O, 4 * NPK * PK], [1, O]]),
                    ap(ebt, [[O, 4 * NPK * PK], [1, O]]),
                    ap(rsum, [[1, 4 * NPK * PK], [0, O]]),
                )
                mark("gp", cT[hb][:, :1])

            # ---- emission: xT0, A0(t0), xT1, A1(t0), B0, B1, then t=1,2 ----
            build_xt(0)
            phase_A(0, 0)
            build_xt(1)
            phase_A(1, 0)
            phase_B(0, 0)
            phase_B(1, 0)
            for t in range(1, NITER):
                phase_A(0, t)
                phase_A(1, t)
                phase_B(0, t)
                phase_B(1, t)

    if split_waits:
        _split_fat_waits(nc)
    return nc


def _split_fat_waits(nc, maxw=1):
    """Walrus caps sync waits per instruction; split overflow onto extra
    same-engine Drain instructions inserted just before the offender."""
    nsplit = 0
    for blk in nc.m.functions[0].blocks:
        new_insts = []
        for inst in blk.instructions:
            si = getattr(inst, "sync_info", None)
            w = list(si.on_wait) if si is not None and si.on_wait else []
            if len(w) > maxw:
                for k in range(0, len(w) - maxw, maxw):
                    d = mybir.InstDrain(name="I-waitsplit-%d" % nsplit,
                                        ins=[], outs=[])
                    nsplit += 1
                    d.engine = inst.engine
                    d.sync_info = mybir.SyncInfo(on_wait=w[k:k + maxw],
                                                 on_update=[])
                    new_insts.append(d)
                si.on_wait = w[len(w) - maxw:]
            new_insts.append(inst)
        blk.instructions[:] = new_insts
    return nc


_NC_CACHE = None


def make_cst(Wn):
    """fp32 constant blob [128, CSTN] matching the device-side layout."""
    cst = np.zeros((P, CSTN), np.float32)
    # W [h, of] -> [h%128, (hc, of)]
    cst[:, CW:CW + 2 * OF] = Wn.reshape(2, P, OF).transpose(1, 0, 2).reshape(P, 2 * OF)
    # WT [of, h] -> [of%128, (m, h)]
    cst[:, CWT:CWT + 4 * H] = (
        Wn.T.reshape(4, P, H).transpose(1, 0, 2).reshape(P, 4 * H))
    cst[:, CID:CID + P] = np.eye(P, dtype=np.float32)
    for p in range(P):
        o = p % O
        cst[p, CMC + o * F:CMC + (o + 1) * F] = 1.0
    cst[np.arange(P), CS4 + np.arange(P) // O] = 1.0
    for p in range(P):
        for m in range(4):
            cst[p, CBM + m * O + 8 * m + p // F] = 1.0
    return cst


def make_cstb():
    """bf16 constant blob [128, CBN]: identity + uniform 1/32."""
    cb = np.zeros((P, CBN), ml_dtypes.bfloat16)
    cb[:, CBID:CBID + P] = np.eye(P, dtype=ml_dtypes.bfloat16)
    cb[:, CBC0:CBC0 + O] = ml_dtypes.bfloat16(1.0 / O)
    return cb


def make_in_maps(x, W):
    x = np.asarray(x, dtype=np.float32)
    xb = np.ascontiguousarray(x.astype(ml_dtypes.bfloat16))
    Wn = np.ascontiguousarray(np.asarray(W, dtype=np.float32).reshape(H, OF))
    cst = make_cst(Wn)
    cstb = make_cstb()
    xs = xb.reshape(NCORES, S, I, H)
    return [
        {"x": np.ascontiguousarray(xs[c]), "cst": cst, "cstb": cstb}
        for c in range(NCORES)
    ]


def kernel(x: np.ndarray, W: np.ndarray) -> np.ndarray:
    global _NC_CACHE
    if _NC_CACHE is None:
        _NC_CACHE = build_program()
    in_maps = make_in_maps(x, W)
    res = run_bass_kernel_spmd(_NC_CACHE, in_maps, core_ids=list(range(NCORES)))
    out = np.stack([res.results[c]["out"] for c in range(NCORES)])
    return out.reshape(B, O, F)


# revision 94
# speedup vs baseline: 1.0983x; 1.0983x over previous
"""Trainium2 Bass kernel for nn_Capsule (capsule routing with dynamic routing).

reference: u = x @ W  (per-sample [512,256]@[256,512] -> [512, (32 o, 16 f)])
           b=0; 3x { c = softmax_o(b); v[o,f] = sum_i c[o,i] u[i,(o,f)];
                     v = squash(v); b[o,i] = sum_f v[o,f] u[i,(o,f)] }
           return v  [B, 32, 16]

Key algebraic restructuring (u is NEVER materialized):
  v_raw[o,f] = sum_i c[o,i] u[i,(o,f)]  =  diag-extract[ (c @ x) @ W ]
      yT[h,(s,o)] = x-chunk stationary @ cT moving   (PE bf16, 32-wide)
      vfull = y @ W  (PE fp32r: yT stationary, W natural moving)
      v_raw = mask * vfull, then per-sample partition-sum via indicator matmul
  b[o,i] = sum_f v[o,f] u[i,(o,f)] = sum_h z[o,h] x[i,h]
      zT[h,(s,o)] = WT-chunk stationary @ VmatT moving (PE fp32r, dst-0)
      bT[i,(s,o)] = xT-chunk stationary @ zT moving    (PE bf16, 32-wide)
  softmax over o directly on bT [i-partition, o-free]; cT feeds next yT.

Dtype split: fp32r (fp32 bits, PE fast path, dst partition 0 only) for
vfull / vr / flipped-z; bf16 for x / xT / cT / zT / exp(b) and the flipped
y and b phases. x is cast to bf16 on the HOST (halves HBM traffic).

The two half-batches are interleaved per routing iteration (A=aggregation
phase, B=agreement phase: emit A0 A1 B0 B1) so each half-batch's serial
squash/softmax latency hides under the other's PE work, and the Activation
engine runs Sqrt x2 then Exp x2 back-to-back (2 table loads per iteration).
"""

import numpy as np
import ml_dtypes

import concourse.bass as bass
import concourse.tile as tile
from concourse import mybir
from concourse.bass_utils import run_bass_kernel_spmd

F32 = mybir.dt.float32
R32 = mybir.dt.float32r
BF16 = mybir.dt.bfloat16
I32 = mybir.dt.int32
AF = mybir.ActivationFunctionType
AX = mybir.AxisListType

B, I, H = 128, 512, 256
O, F = 32, 16
OF = O * F  # 512
NCORES = 8
S = B // NCORES      # 16 samples per core
NHB = 2              # half-batches per core
NPK = 2              # packs per half-batch
PK = 4               # samples per pack
NITER = 3
P = 128

# fp32 constant blob layout (one DMA, per-partition element offsets)
CW = 0                  # W  [h%128, (hc 2, of 512)]
CWT = CW + 2 * OF       # WT [of%128, (m 4, h 256)]
CID = CWT + 4 * H       # identity [128, 128]
CMC = CID + P           # diag mask [128, 512]
CS4 = CMC + OF          # sample-sum indicator [128, 4]
CBM = CS4 + PK          # Vmat block masks [128, (m 4, j 32)]
CSTN = CBM + 4 * O

# bf16 constant blob
CBID = 0                # identity [128, 128] bf16
CBC0 = CBID + P         # uniform 1/32 [128, 32] bf16
CBN = CBC0 + O


def ap(t, dims, off=0):
    """AP over tile/handle `t`: keep partition dim, explicit free dims."""
    a = t if isinstance(t, bass.AP) else t[:]
    return bass.AP(tensor=a.tensor, offset=a.offset + off,
                   ap=[list(a.ap[0])] + [list(d) for d in dims])


def fview(a):
    """Alias a float32r AP as plain fp32 (same bytes) for transposes/DVE."""
    t = a.tensor
    if t.dtype != R32:
        return a
    t2 = bass.SBTensorHandle(name=t.name, shape=t.shape, dtype=F32,
                             base_partition=t.base_partition,
                             manual_sbuf_range=t.manual_sbuf_range,
                             manual_base_name=t.manual_base_name)
    return bass.AP(tensor=t2, offset=a.offset,
                   ap=[list(d) for d in a.ap])


def aview(a, dt):
    """Alias an AP as another same-width dtype (bit reinterpret, READ only:
    tile write tracking does not see writes through an aliased handle)."""
    t = a.tensor
    t2 = bass.SBTensorHandle(name=t.name, shape=t.shape, dtype=dt,
                             base_partition=t.base_partition,
                             manual_sbuf_range=t.manual_sbuf_range,
                             manual_base_name=t.manual_base_name)
    return bass.AP(tensor=t2, offset=a.offset,
                   ap=[list(d) for d in a.ap])


def dram_ap(handle, dims, off=0):
    """AP over DRAM handle with fully explicit dims (first = partition)."""
    a = handle[:]
    return bass.AP(tensor=a.tensor, offset=a.offset + off,
                   ap=[list(d) for d in dims])


def build_program(split_waits=True, loop_n=None):
    """loop_n: wrap the whole body in a hardware For_i loop (timing runs)."""
    import contextlib

    nc = bass.Bass("TRN2", target_bir_lowering=False)

    x_d = nc.dram_tensor("x", [S, I, H], BF16, kind="ExternalInput")
    cst_d = nc.dram_tensor("cst", [P, CSTN], R32, kind="ExternalInput")
    cstb_d = nc.dram_tensor("cstb", [P, CBN], BF16, kind="ExternalInput")
    out_d = nc.dram_tensor("out", [S, OF], F32, kind="ExternalOutput")

    with tile.TileContext(nc) as tc:
        with (
            tc.tile_pool(name="consts", bufs=1) as consts,
            tc.tile_pool(name="xpool", bufs=4) as xpool,
            tc.tile_pool(name="xtpool", bufs=4) as xtpool,
            tc.tile_pool(name="work", bufs=3) as work,
            tc.tile_pool(name="sm", bufs=8) as sm,
            tc.tile_pool(name="ps2k", bufs=5, space="PSUM") as ps2k,
            tc.tile_pool(name="ps_xt", bufs=2, space="PSUM") as ps_xt,
            tc.tile_pool(name="ps_anch", bufs=1, space="PSUM") as ps_anch,
            tc.For_i(0, loop_n, 1) if loop_n else contextlib.nullcontext(),
        ):
            # ---- constants ----
            cstb = consts.tile([P, CBN], BF16)
            nc.sync.dma_start(out=cstb[:], in_=cstb_d[:])
            idb_sb = cstb[:, CBID:CBID + P]      # identity bf16
            c0b_sb = cstb[:, CBC0:CBC0 + O]      # uniform 1/32 bf16

            cst = consts.tile([P, CSTN], R32)
            w_sb = cst[:, CW:CW + 2 * OF]        # [h%128, (hc, of)] R32
            wt_sb = cst[:, CWT:CWT + 4 * H]      # [of%128, (m, h)] R32
            idr_sb = cst[:, CID:CID + P]         # identity (R32 transposes)
            id_sb = fview(idr_sb)                # identity (fp32 transposes)
            mc_sb = fview(cst[:, CMC:CMC + OF])  # diag mask (p%32 == o)
            s4_sb = cst[:, CS4:CS4 + PK]         # s4[p,s] = (p//32 == s) R32
            bm_sb = fview(cst[:, CBM:CBM + 4 * O])  # bm[p,(m,j)]=(j==8m+p//16)

            # PE sync anchors: every datapath instruction carries at most ONE
            # sem wait (walrus).  A 1x1 transpose reading a byte of a dirty
            # foreign-engine tensor makes PE "observe" that engine's clock so
            # later PE instructions need no cross-engine waits.
            anch = ps_anch.tile([P, F], F32)
            # bf16 alias of the same PSUM bytes (even cols: 4B alignment)
            anchb = anch[:].tensor.bitcast(BF16)[:]
            dirty = {}
            acol = [0]
            pending = []

            def mark(key, apv):
                dirty[key] = apv

            def pe_sync(*keys):
                pending.clear()
                for k in keys:
                    if k not in dirty:
                        continue
                    d = dirty.pop(k)
                    dd = fview(d[:1, :1])
                    if dd.tensor.dtype == BF16:
                        a = nc.tensor.transpose(
                            anchb[:1, 2 * acol[0]:2 * acol[0] + 1], dd,
                            idb_sb[:1, :1])
                    else:
                        a = nc.tensor.transpose(
                            anch[:1, acol[0]:acol[0] + 1], dd,
                            id_sb[:1, :1])
                    pending.append(a.ins)
                    acol[0] = (acol[0] + 1) % F

            def _chain(b):
                for a in pending:
                    bass._add_dep_helper(b.ins, a, sync=False,
                                         reason="pe-anchor order")
                return b

            def T(out, in_, ident):
                return _chain(nc.tensor.transpose(out, in_, ident))

            def MM(out, lhsT, rhs, **kw):
                return _chain(nc.tensor.matmul(out, lhsT, rhs, **kw))

            def dep(b, a):
                if a is not None:
                    bass._add_dep_helper(b.ins, a, sync=False,
                                         reason="engine-anchor order")
                return b

            mark("cstb", cstb)

            dscr = sm.tile([PK, PK], F32, tag="dscr")
            nc.vector.memset(dscr[:], 0.0)

            # ---- x loads (bf16, natural [i, h]) ----
            x_sb = {}   # (hb, pk) -> flat [128, (s, ic, h)] = [128, 4096]
            xt_sb = {}  # (hb, pk) -> flat [128, (s, hc, i)] = [128, 4096]
            for hb in range(NHB):
                for pk in range(NPK):
                    samp0 = hb * 8 + pk * 4
                    xs = xpool.tile([P, PK * 4 * H], BF16, tag="x")
                    # per-sample chunks: transposes/y start after 1/4 tile
                    for s in range(PK):
                        nc.sync.dma_start(
                            out=ap(xs, [[H, 4], [1, H]], off=s * 4 * H),
                            in_=dram_ap(x_d, [[H, P], [P * H, 4], [1, H]],
                                        off=(samp0 + s) * I * H),
                        )
                        mark("x%d%d%d" % (hb, pk, s), xs[:, s * 4 * H:
                                                        s * 4 * H + 1])
                    x_sb[(hb, pk)] = xs
                if hb == 0:
                    # consts DMA between the two half-batches' x loads:
                    # cst is first needed at vfull(hb0,t0).
                    nc.sync.dma_start(out=cst[:], in_=cst_d[:])
                    mark("cst", cst)
            # one-time: let DVE observe the const DMA (mc/bm reads)
            dcst_a = nc.vector.tensor_copy(dscr[:1, :1],
                                           fview(cst[:1, :1])).ins

            def build_xt(hb):
                """xT via PE transposes; copies split DVE/Act/Pool."""
                for pk in range(NPK):
                    xs = x_sb[(hb, pk)]
                    xt = xtpool.tile([P, PK * 2 * I], BF16, tag="xt")
                    for s in range(PK):
                        pe_sync("cstb", "x%d%d%d" % (hb, pk, s),
                                "act", "dve", "gp")
                        pxt = ps_xt.tile([P, 2 * I], BF16, tag="pxt")
                        for hc in range(2):
                            for ic in range(4):
                                T(
                                    pxt[:, hc * I + ic * P:
                                        hc * I + (ic + 1) * P],
                                    xs[:, s * 1024 + ic * H + hc * P:
                                           s * 1024 + ic * H + (hc + 1) * P],
                                    idb_sb,
                                )
                        # Act is idle until the first Exp (~35us): give it
                        # most xT copies, freeing DVE for the t0 chain.
                        dst = xt[:, s * 1024: (s + 1) * 1024]
                        if s % 2 == 0:
                            nc.scalar.activation(dst, pxt[:], AF.Copy)
                            mark("act", dst)
                        else:
                            nc.vector.tensor_copy(dst, pxt[:])
                            mark("dve", dst)
                    xt_sb[(hb, pk)] = xt

            cT = {0: None, 1: None}  # per-hb bf16 [128 i%128, (ic, pk, s, o)]
            vstate = {}              # per-hb carry between A and B phases

            def phase_A(hb, t):
                """y/vfull/vr/squash (through vsq): aggregation phase."""
                ve = nc.vector if hb == 0 else nc.gpsimd
                vkey = "dve" if hb == 0 else "gp"
                pvr_of = {}
                if t == 0:
                    # ---- t0 fast path: c uniform 1/32 -> every o gets the
                    # same y row, so v_raw = (xsum/32) @ W needs no diag
                    # extract: xsum via 1-wide matmuls, then a [h,4]-wide
                    # vfull lands v_raw per pack directly. ----
                    pe_sync("cstb", "dve", "act", "gp",
                            *["x%d%d%d" % (hb, pk_, s_)
                              for pk_ in range(NPK) for s_ in range(PK)])
                    ps_y0 = ps2k.tile([P, NPK * PK * 2], F32, tag="p2k")
                    for hc in range(2):
                        for pk in range(NPK):
                            for s in range(PK):
                                for ic in range(4):
                                    MM(
                                        ps_y0[:, hc * 8 + pk * PK + s:
                                              hc * 8 + pk * PK + s + 1],
                                        ap(x_sb[hb, pk], [[1, P]],
                                           off=s * 1024 + ic * H + hc * P),
                                        c0b_sb[:, :1],
                                        start=(ic == 0),
                                        stop=(ic == 3),
                                    )
                    yt0_sb = work.tile([P, NPK * PK * 2], R32, tag="yt0")
                    nc.vector.tensor_copy(yt0_sb[:], ps_y0[:])
                    mark("dve", yt0_sb)
                    pe_sync("dve", "cst")
                    for pk in range(NPK):
                        pvr = ps2k.tile([PK, OF], F32, tag="p2k")
                        for hc in range(2):
                            MM(
                                pvr[:],
                                ap(yt0_sb, [[1, PK]], off=hc * 8 + pk * PK),
                                ap(w_sb, [[1, OF]], off=hc * OF),
                                start=(hc == 0),
                                stop=(hc == 1),
                            )
                        pvr_of[pk] = pvr
                else:
                    # ---- yT[h,(hc,pk,s,o)] = x-stationary @ cT (bf16) ----
                    pe_sync("cstb", "dve", "act", "gp",
                            *["x%d%d%d" % (hb, pk_, s_)
                              for pk_ in range(NPK) for s_ in range(PK)])
                    ps_yt = ps2k.tile([P, NPK * H], F32, tag="p2k")
                    for hc in range(2):
                        for pk in range(NPK):
                            for s in range(PK):
                                for ic in range(4):
                                    MM(
                                        ps_yt[:, hc * 256 + pk * P + s * O:
                                              hc * 256 + pk * P + s * O + O],
                                        ap(x_sb[hb, pk], [[1, P]],
                                           off=s * 1024 + ic * H + hc * P),
                                        ap(cT[hb], [[1, O]],
                                           off=ic * 256 + pk * P + s * O),
                                        start=(ic == 0),
                                        stop=(ic == 3),
                                    )
                    yt_sb = work.tile([P, NPK * H], R32, tag="ytsb")
                    nc.scalar.activation(yt_sb[:], ps_yt[:], AF.Copy)
                    mark("act", yt_sb)

                    # ---- vfull = y @ W (fp32r): [(pk,) 4s*32o', (o,f)] ----
                    pe_sync("dve", "cst")
                    msk_sb = work.tile([P, NPK * OF], R32, tag="bigsb")
                    for pk in range(NPK):
                        pvf = ps2k.tile([P, OF], F32, tag="p2k")
                        for hc in range(2):
                            MM(
                                pvf[:],
                                ap(yt_sb, [[1, P]], off=hc * 256 + pk * P),
                                ap(w_sb, [[1, OF]], off=hc * OF),
                                start=(hc == 0),
                                stop=(hc == 1),
                            )
                        # diag extract mask (release vf bank early per pk)
                        dep(nc.vector.tensor_mul(
                            ap(msk_sb, [[1, OF]], off=pk * OF),
                            pvf[:],
                            ap(mc_sb, [[1, OF]]),
                        ), dcst_a)
                        mark("dve", msk_sb[:, pk * OF:pk * OF + 1])

                # ---- per-sample partition sum (fp32r indicator), per pk ----
                pe_sync("dve")
                vr_sb = work.tile([PK, NPK * OF], F32, tag="vrsb")
                sq_sb = work.tile([PK, NPK * OF], F32, tag="sqsb")
                mag = sm.tile([PK, NPK * O], F32, tag="mag")
                red_ins = []
                for pk in range(NPK):
                    if t == 0:
                        pvr = pvr_of[pk]
                    else:
                        pvr = ps2k.tile([PK, OF], F32, tag="p2k")
                        MM(
                            pvr[:],
                            s4_sb,
                            msk_sb[:, pk * OF:(pk + 1) * OF],
                            start=True, stop=True,
                        )
                    # DVE may read at most one PSUM operand: land vr in
                    # SBUF once, square/scale from there. Copy on Act: the
                    # A-phase Copy block and B-phase Exp block each reload
                    # the act table once per t.
                    nc.scalar.activation(
                        ap(vr_sb, [[1, OF]], off=pk * OF), pvr[:], AF.Copy)
                    mark("act", vr_sb[:, pk * OF:pk * OF + 1])
                    ve.tensor_mul(
                        ap(sq_sb, [[1, OF]], off=pk * OF),
                        ap(vr_sb, [[1, OF]], off=pk * OF),
                        ap(vr_sb, [[1, OF]], off=pk * OF))
                red_ins.append(nc.vector.reduce_sum(
                    out=mag[:],
                    in_=ap(sq_sb, [[F, NPK * O], [1, F]]),
                    axis=AX.X,
                ).ins)

                # ---- squash: factor = sqrt(mag)/(1+mag), sqrt via rsqrt
                # bit-trick + 2 Newton steps (keeps the Act table on Exp).
                # hb0's scalar chain runs on DVE, hb1's on Pool so the two
                # half-batches don't contend.
                sh_t = sm.tile([PK, NPK * O], I32, tag="sh")
                # int shift is DVE-only (Pool ALU lacks it)
                sh_i = nc.vector.tensor_scalar(
                    out=sh_t[:], in0=aview(mag[:], I32), scalar1=1,
                    scalar2=None,
                    op0=mybir.AluOpType.arith_shift_right)
                for ri_ in red_ins:  # alias reads are not dep-tracked
                    bass._add_dep_helper(sh_i.ins, ri_, sync=True,
                                         reason="aliased mag read")
                r0_t = sm.tile([PK, NPK * O], I32, tag="s0")
                nc.vector.tensor_scalar(
                    out=r0_t[:], in0=sh_t[:], scalar1=-1,
                    scalar2=0x5F3759DF,
                    op0=mybir.AluOpType.mult, op1=mybir.AluOpType.add)
                r = aview(r0_t[:], F32)
                for _ in range(1):  # Newton: r = r*(1.5 - 0.5*m*r^2)
                    a = sm.tile([PK, NPK * O], F32, tag="nta")
                    ve.tensor_mul(a[:], mag[:], r)
                    h = sm.tile([PK, NPK * O], F32, tag="nth")
                    ve.tensor_mul(h[:], a[:], r)
                    w = sm.tile([PK, NPK * O], F32, tag="ntw")
                    ve.tensor_scalar(
                        out=w[:], in0=h[:], scalar1=-0.5, scalar2=1.5,
                        op0=mybir.AluOpType.mult, op1=mybir.AluOpType.add)
                    r2 = sm.tile([PK, NPK * O], F32, tag="ntr")
                    ve.tensor_mul(r2[:], r, w[:])
                    r = r2[:]
                s0 = sm.tile([PK, NPK * O], F32, tag="s0f")
                ve.tensor_mul(s0[:], mag[:], r)  # sqrt = m * rsqrt
                onep = sm.tile([PK, NPK * O], F32, tag="onep")
                ve.tensor_scalar_add(onep[:], mag[:], 1.0)
                rec = sm.tile([PK, NPK * O], F32, tag="rec")
                nc.vector.reciprocal(rec[:], onep[:])
                factor = sm.tile([PK, NPK * O], F32, tag="fac")
                ve.tensor_mul(factor[:], s0[:], rec[:])
                vsq = work.tile([PK, NPK * OF], F32, tag="vsq", bufs=4)
                for pk in range(NPK):
                    ve.tensor_mul(
                        ap(vsq, [[F, O], [1, F]], off=pk * OF),
                        ap(vr_sb, [[F, O], [1, F]], off=pk * OF),
                        ap(factor, [[1, O], [0, F]], off=pk * O),
                    )
                mark(vkey, vsq)
                vstate[hb] = vsq

            def phase_B(hb, t):
                """agreement phase: vT/vp2/zT/bT/softmax (skip on last iter)."""
                vsq = vstate[hb]

                if t == NITER - 1:
                    dq = nc.sync if hb == 0 else nc.scalar
                    dq.dma_start(
                        out=dram_ap(out_d, [[OF, PK], [PK * OF, NPK], [1, OF]],
                                    off=hb * 8 * OF),
                        in_=ap(vsq, [[OF, NPK], [1, OF]]),
                    )
                    return

                # ---- vT chunks: [(o8,f16)%128, (pk, m, s)] ----
                pe_sync("dve", "act")
                ps_vt = ps2k.tile([P, NPK * 4 * PK], F32, tag="p2k")
                for pk in range(NPK):
                    for m in range(4):
                        T(
                            ps_vt[:, (pk * 4 + m) * PK:(pk * 4 + m + 1) * PK],
                            vsq[:, pk * OF + m * P: pk * OF + (m + 1) * P],
                            id_sb[:PK, :PK],
                        )
                vt_sb = work.tile([P, NPK * 4 * PK], F32, tag="vtsb")
                nc.vector.tensor_copy(vt_sb[:], ps_vt[:])

                # ---- VmatT blocks: vp2[p,(m,pk,s,o)] = vtT * blockmask ----
                vp2_sb = work.tile([P, 4 * NPK * PK * O], R32, tag="vp")
                ve2 = nc.gpsimd
                for m in range(4):
                    dep(ve2.tensor_mul(
                        ap(vp2_sb, [[PK * O, NPK], [O, PK], [1, O]],
                           off=m * NPK * PK * O),
                        ap(vt_sb, [[4 * PK, NPK], [1, PK], [0, O]],
                           off=m * PK),
                        ap(bm_sb, [[0, NPK], [0, PK], [1, O]],
                           off=m * O),
                    ), dcst_a)
                mark("gp",
                     vp2_sb[:, 3 * NPK * PK * O:3 * NPK * PK * O + 1])

                # ---- zT = WT-chunk stationary @ VmatT (fp32r, dst 0):
                #      [h%128, (hc, pk, s, o)] ----
                pe_sync("dve", "act", "gp")
                ps_zt = ps2k.tile([P, NPK * H], F32, tag="p2k")
                for hc in range(2):
                    for m in range(4):
                        MM(
                            ps_zt[:, hc * NPK * P:(hc + 1) * NPK * P],
                            ap(wt_sb, [[1, P]], off=m * H + hc * P),
                            ap(vp2_sb, [[1, NPK * P]], off=m * NPK * P),
                            start=(m == 0),
                            stop=(m == 3),
                        )
                zt_sb = work.tile([P, NPK * H], BF16, tag="ztsb")
                nc.vector.tensor_copy(zt_sb[:], ps_zt[:])
                mark("dve", zt_sb)

                # ---- bT[i,(ic),(pk,s,o)] = xT-chunk stationary @ zT (bf16),
                #      two 2KB PSUM halves so exp(half0) overlaps half1 ----
                pe_sync("dve")
                ebt = work.tile([P, 4 * NPK * P], BF16, tag="ebt")
                for half in range(2):
                    ps_bt = ps2k.tile([P, NPK * P * 2], F32, tag="p2k")
                    for ic2 in range(2):
                        ic = half * 2 + ic2
                        for pk in range(NPK):
                            for s in range(PK):
                                for hc in range(2):
                                    MM(
                                        ps_bt[:, ic2 * 256 + pk * P + s * O:
                                              ic2 * 256 + pk * P + s * O + O],
                                        ap(xt_sb[hb, pk], [[1, P]],
                                           off=s * 1024 + hc * I + ic * P),
                                        ap(zt_sb, [[1, O]],
                                           off=hc * NPK * P + pk * P + s * O),
                                        start=(hc == 0),
                                        stop=(hc == 1),
                                    )
                    # ---- softmax exp (b in +-5: exp w/o max-sub) ----
                    nc.scalar.activation(
                        ebt[:, half * 512:(half + 1) * 512],
                        ps_bt[:], AF.Exp)
                mark("act", ebt)
                ssum = sm.tile([P, 4 * NPK * PK], F32, tag="ssum")
                nc.vector.reduce_sum(
                    out=ssum[:],
                    in_=ap(ebt, [[O, 4 * NPK * PK], [1, O]]),
                    axis=AX.X,
                )
                rsum = sm.tile([P, 4 * NPK * PK], F32, tag="rsum")
                nc.vector.reciprocal(rsum[:], ssum[:])
                cT[hb] = work.tile([P, 4 * NPK * P], BF16, tag="ct%d" % hb,
                                   name="ct_t")
                ve3 = nc.gpsimd
                ve3.tensor_mul(
                    ap(cT[hb], [[O, 4 * NPK * PK], [1, O]]),
                    ap(ebt, [[O, 4 * NPK * PK], [1, O]]),
                    ap(rsum, [[1, 4 * NPK * PK], [0, O]]),
                )
                mark("gp", cT[hb][:, :1])

            # ---- emission: xT0, A0(t0), xT1, A1(t0), B0, B1, then t=1,2 ----
            build_xt(0)
            phase_A(0, 0)
            build_xt(1)
            phase_A(1, 0)
            phase_B(0, 0)
            phase_B(1, 0)
            for t in range(1, NITER):
                phase_A(0, t)
                phase_A(1, t)
                phase_B(0, t)
                phase_B(1, t)

    if split_waits:
        _split_fat_waits(nc)
    return nc


def _split_fat_waits(nc, maxw=1):
    """Walrus caps sync waits per instruction; split overflow onto extra
    same-engine Drain instructions inserted just before the offender."""
    nsplit = 0
    for blk in nc.m.functions[0].blocks:
        new_insts = []
        for inst in blk.instructions:
            si = getattr(inst, "sync_info", None)
            w = list(si.on_wait) if si is not None and si.on_wait else []
            if len(w) > maxw:
                for k in range(0, len(w) - maxw, maxw):
                    d = mybir.InstDrain(name="I-waitsplit-%d" % nsplit,
                                        ins=[], outs=[])
                    nsplit += 1
                    d.engine = inst.engine
                    d.sync_info = mybir.SyncInfo(on_wait=w[k:k + maxw],
                                                 on_update=[])
                    new_insts.append(d)
                si.on_wait = w[len(w) - maxw:]
            new_insts.append(inst)
        blk.instructions[:] = new_insts
    return nc


_NC_CACHE = None


def make_cst(Wn):
    """fp32 constant blob [128, CSTN] matching the device-side layout."""
    cst = np.zeros((P, CSTN), np.float32)
    # W [h, of] -> [h%128, (hc, of)]
    cst[:, CW:CW + 2 * OF] = Wn.reshape(2, P, OF).transpose(1, 0, 2).reshape(P, 2 * OF)
    # WT [of, h] -> [of%128, (m, h)]
    cst[:, CWT:CWT + 4 * H] = (
        Wn.T.reshape(4, P, H).transpose(1, 0, 2).reshape(P, 4 * H))
    cst[:, CID:CID + P] = np.eye(P, dtype=np.float32)
    for p in range(P):
        o = p % O
        cst[p, CMC + o * F:CMC + (o + 1) * F] = 1.0
    cst[np.arange(P), CS4 + np.arange(P) // O] = 1.0
    for p in range(P):
        for m in range(4):
            cst[p, CBM + m * O + 8 * m + p // F] = 1.0
    return cst


def make_cstb():
    """bf16 constant blob [128, CBN]: identity + uniform 1/32."""
    cb = np.zeros((P, CBN), ml_dtypes.bfloat16)
    cb[:, CBID:CBID + P] = np.eye(P, dtype=ml_dtypes.bfloat16)
    cb[:, CBC0:CBC0 + O] = ml_dtypes.bfloat16(1.0 / O)
    return cb


def make_in_maps(x, W):
    x = np.asarray(x, dtype=np.float32)
    xb = np.ascontiguousarray(x.astype(ml_dtypes.bfloat16))
    Wn = np.ascontiguousarray(np.asarray(W, dtype=np.float32).reshape(H, OF))
    cst = make_cst(Wn)
    cstb = make_cstb()
    xs = xb.reshape(NCORES, S, I, H)
    return [
        {"x": np.ascontiguousarray(xs[c]), "cst": cst, "cstb": cstb}
        for c in range(NCORES)
    ]


def kernel(x: np.ndarray, W: np.ndarray) -> np.ndarray:
    global _NC_CACHE
    if _NC_CACHE is None:
        _NC_CACHE = build_program()
    in_maps = make_in_maps(x, W)
    res = run_bass_kernel_spmd(_NC_CACHE, in_maps, core_ids=list(range(NCORES)))
    out = np.stack([res.results[c]["out"] for c in range(NCORES)])
    return out.reshape(B, O, F)


# revision 96
# speedup vs baseline: 1.4944x; 1.3607x over previous
"""Trainium2 Bass kernel for nn_Capsule (capsule routing with dynamic routing).

reference: u = x @ W  (per-sample [512,256]@[256,512] -> [512, (32 o, 16 f)])
           b=0; 3x { c = softmax_o(b); v[o,f] = sum_i c[o,i] u[i,(o,f)];
                     v = squash(v); b[o,i] = sum_f v[o,f] u[i,(o,f)] }
           return v  [B, 32, 16]

Key algebraic restructuring (u is NEVER materialized):
  v_raw[o,f] = sum_i c[o,i] u[i,(o,f)]  =  diag-extract[ (c @ x) @ W ]
      yT[h,(s,o)] = x-chunk stationary @ cT moving   (PE bf16, 32-wide)
      vfull = y @ W  (PE fp32r: yT stationary, W natural moving)
      v_raw = mask * vfull, then per-sample partition-sum via indicator matmul
  b[o,i] = sum_f v[o,f] u[i,(o,f)] = sum_h z[o,h] x[i,h]
      zT[h,(s,o)] = WT-chunk stationary @ VmatT moving (PE fp32r, dst-0)
      bT[i,(s,o)] = xT-chunk stationary @ zT moving    (PE bf16, 32-wide)
  softmax over o directly on bT [i-partition, o-free]; cT feeds next yT.

Dtype split: fp32r (fp32 bits, PE fast path, dst partition 0 only) for
vfull / vr / flipped-z; bf16 for x / xT / cT / zT / exp(b) and the flipped
y and b phases. x is cast to bf16 on the HOST (halves HBM traffic).

The two half-batches are interleaved per routing iteration (A=aggregation
phase, B=agreement phase: emit A0 A1 B0 B1) so each half-batch's serial
squash/softmax latency hides under the other's PE work, and the Activation
engine runs Sqrt x2 then Exp x2 back-to-back (2 table loads per iteration).
"""

import numpy as np
import ml_dtypes

import concourse.bass as bass
import concourse.tile as tile
from concourse import mybir
from concourse.bass_utils import run_bass_kernel_spmd

F32 = mybir.dt.float32
R32 = mybir.dt.float32r
BF16 = mybir.dt.bfloat16
I32 = mybir.dt.int32
AF = mybir.ActivationFunctionType
AX = mybir.AxisListType

B, I, H = 128, 512, 256
O, F = 32, 16
OF = O * F  # 512
NCORES = 8
S = B // NCORES      # 16 samples per core
NHB = 2              # half-batches per core
NPK = 2              # packs per half-batch
PK = 4               # samples per pack
NITER = 3
P = 128

# fp32 constant blob layout (one DMA, per-partition element offsets)
CW = 0                  # W  [h%128, (hc 2, of 512)]
CWT = CW + 2 * OF       # WT [of%128, (m 4, h 256)]
CID = CWT + 4 * H       # identity [128, 128]
CMC = CID + P           # diag mask [128, 512]
CS4 = CMC + OF          # sample-sum indicator [128, 4]
CBM = CS4 + PK          # Vmat block masks [128, (m 4, j 32)]
CSTN = CBM + 4 * O

# bf16 constant blob
CBID = 0                # identity [128, 128] bf16
CBC0 = CBID + P         # uniform 1/32 [128, 32] bf16
CBN = CBC0 + O


def ap(t, dims, off=0):
    """AP over tile/handle `t`: keep partition dim, explicit free dims."""
    a = t if isinstance(t, bass.AP) else t[:]
    return bass.AP(tensor=a.tensor, offset=a.offset + off,
                   ap=[list(a.ap[0])] + [list(d) for d in dims])


def fview(a):
    """Alias a float32r AP as plain fp32 (same bytes) for transposes/DVE."""
    t = a.tensor
    if t.dtype != R32:
        return a
    t2 = bass.SBTensorHandle(name=t.name, shape=t.shape, dtype=F32,
                             base_partition=t.base_partition,
                             manual_sbuf_range=t.manual_sbuf_range,
                             manual_base_name=t.manual_base_name)
    return bass.AP(tensor=t2, offset=a.offset,
                   ap=[list(d) for d in a.ap])


def aview(a, dt):
    """Alias an AP as another same-width dtype (bit reinterpret, READ only:
    tile write tracking does not see writes through an aliased handle)."""
    t = a.tensor
    t2 = bass.SBTensorHandle(name=t.name, shape=t.shape, dtype=dt,
                             base_partition=t.base_partition,
                             manual_sbuf_range=t.manual_sbuf_range,
                             manual_base_name=t.manual_base_name)
    return bass.AP(tensor=t2, offset=a.offset,
                   ap=[list(d) for d in a.ap])


def dram_ap(handle, dims, off=0):
    """AP over DRAM handle with fully explicit dims (first = partition)."""
    a = handle[:]
    return bass.AP(tensor=a.tensor, offset=a.offset + off,
                   ap=[list(d) for d in dims])


def build_program(split_waits=True, loop_n=None):
    """loop_n: wrap the whole body in a hardware For_i loop (timing runs)."""
    import contextlib

    nc = bass.Bass("TRN2", target_bir_lowering=False)

    x_d = nc.dram_tensor("x", [S, I, H], BF16, kind="ExternalInput")
    cst_d = nc.dram_tensor("cst", [P, CSTN], R32, kind="ExternalInput")
    cstb_d = nc.dram_tensor("cstb", [P, CBN], BF16, kind="ExternalInput")
    out_d = nc.dram_tensor("out", [S, OF], F32, kind="ExternalOutput")

    with tile.TileContext(nc) as tc:
        with (
            tc.tile_pool(name="consts", bufs=1) as consts,
            tc.tile_pool(name="xpool", bufs=4) as xpool,
            tc.tile_pool(name="xtpool", bufs=4) as xtpool,
            tc.tile_pool(name="work", bufs=3) as work,
            tc.tile_pool(name="sm", bufs=8) as sm,
            tc.tile_pool(name="ps2k", bufs=5, space="PSUM") as ps2k,
            tc.tile_pool(name="ps_xt", bufs=2, space="PSUM") as ps_xt,
            tc.tile_pool(name="ps_anch", bufs=1, space="PSUM") as ps_anch,
            tc.For_i(0, loop_n, 1) if loop_n else contextlib.nullcontext(),
        ):
            # ---- constants ----
            cstb = consts.tile([P, CBN], BF16)
            nc.sync.dma_start(out=cstb[:], in_=cstb_d[:])
            idb_sb = cstb[:, CBID:CBID + P]      # identity bf16
            c0b_sb = cstb[:, CBC0:CBC0 + O]      # uniform 1/32 bf16

            cst = consts.tile([P, CSTN], R32)
            w_sb = cst[:, CW:CW + 2 * OF]        # [h%128, (hc, of)] R32
            wt_sb = cst[:, CWT:CWT + 4 * H]      # [of%128, (m, h)] R32
            idr_sb = cst[:, CID:CID + P]         # identity (R32 transposes)
            id_sb = fview(idr_sb)                # identity (fp32 transposes)
            mc_sb = fview(cst[:, CMC:CMC + OF])  # diag mask (p%32 == o)
            s4_sb = cst[:, CS4:CS4 + PK]         # s4[p,s] = (p//32 == s) R32
            bm_sb = fview(cst[:, CBM:CBM + 4 * O])  # bm[p,(m,j)]=(j==8m+p//16)

            # PE sync anchors: every datapath instruction carries at most ONE
            # sem wait (walrus).  A 1x1 transpose reading a byte of a dirty
            # foreign-engine tensor makes PE "observe" that engine's clock so
            # later PE instructions need no cross-engine waits.
            anch = ps_anch.tile([P, F], F32)
            # bf16 alias of the same PSUM bytes (even cols: 4B alignment)
            anchb = anch[:].tensor.bitcast(BF16)[:]
            dirty = {}
            acol = [0]
            pending = []

            def mark(key, apv):
                dirty[key] = apv

            def pe_sync(*keys):
                pending.clear()
                for k in keys:
                    if k not in dirty:
                        continue
                    d = dirty.pop(k)
                    dd = fview(d[:1, :1])
                    if dd.tensor.dtype == BF16:
                        a = nc.tensor.transpose(
                            anchb[:1, 2 * acol[0]:2 * acol[0] + 1], dd,
                            idb_sb[:1, :1])
                    else:
                        a = nc.tensor.transpose(
                            anch[:1, acol[0]:acol[0] + 1], dd,
                            id_sb[:1, :1])
                    pending.append(a.ins)
                    acol[0] = (acol[0] + 1) % F

            def _chain(b):
                for a in pending:
                    bass._add_dep_helper(b.ins, a, sync=False,
                                         reason="pe-anchor order")
                return b

            def T(out, in_, ident):
                return _chain(nc.tensor.transpose(out, in_, ident))

            def MM(out, lhsT, rhs, **kw):
                return _chain(nc.tensor.matmul(out, lhsT, rhs, **kw))

            def dep(b, a):
                if a is not None:
                    bass._add_dep_helper(b.ins, a, sync=False,
                                         reason="engine-anchor order")
                return b

            mark("cstb", cstb)

            dscr = sm.tile([PK, PK], F32, tag="dscr")
            nc.vector.memset(dscr[:], 0.0)

            # ---- x loads (bf16, natural [i, h]) ----
            x_sb = {}   # (hb, pk) -> flat [128, (s, ic, h)] = [128, 4096]
            xt_sb = {}  # (hb, pk) -> flat [128, (s, hc, i)] = [128, 4096]
            for hb in range(NHB):
                for pk in range(NPK):
                    samp0 = hb * 8 + pk * 4
                    xs = xpool.tile([P, PK * 4 * H], BF16, tag="x")
                    # per-sample chunks: transposes/y start after 1/4 tile
                    for s in range(PK):
                        nc.sync.dma_start(
                            out=ap(xs, [[H, 4], [1, H]], off=s * 4 * H),
                            in_=dram_ap(x_d, [[H, P], [P * H, 4], [1, H]],
                                        off=(samp0 + s) * I * H),
                        )
                        mark("x%d%d%d" % (hb, pk, s), xs[:, s * 4 * H:
                                                        s * 4 * H + 1])
                    x_sb[(hb, pk)] = xs
                if hb == 0:
                    # consts DMA between the two half-batches' x loads:
                    # cst is first needed at vfull(hb0,t0).
                    nc.sync.dma_start(out=cst[:], in_=cst_d[:])
                    mark("cst", cst)
            # one-time: let DVE observe the const DMA (mc/bm reads)
            dcst_a = nc.vector.tensor_copy(dscr[:1, :1],
                                           fview(cst[:1, :1])).ins

            def build_xt(hb):
                """xT via PE transposes; copies split DVE/Act/Pool."""
                for pk in range(NPK):
                    xs = x_sb[(hb, pk)]
                    xt = xtpool.tile([P, PK * 2 * I], BF16, tag="xt")
                    for s in range(PK):
                        pe_sync("cstb", "x%d%d%d" % (hb, pk, s),
                                "act", "dve", "gp")
                        pxt = ps_xt.tile([P, 2 * I], BF16, tag="pxt")
                        for hc in range(2):
                            for ic in range(4):
                                T(
                                    pxt[:, hc * I + ic * P:
                                        hc * I + (ic + 1) * P],
                                    xs[:, s * 1024 + ic * H + hc * P:
                                           s * 1024 + ic * H + (hc + 1) * P],
                                    idb_sb,
                                )
                        # Act is idle until the first Exp (~35us): give it
                        # most xT copies, freeing DVE for the t0 chain.
                        dst = xt[:, s * 1024: (s + 1) * 1024]
                        if s % 2 == 0:
                            nc.scalar.activation(dst, pxt[:], AF.Copy)
                            mark("act", dst)
                        else:
                            nc.vector.tensor_copy(dst, pxt[:])
                            mark("dve", dst)
                    xt_sb[(hb, pk)] = xt

            cT = {0: None, 1: None}  # per-hb bf16 [128 i%128, (ic, pk, s, o)]
            vstate = {}              # per-hb carry between A and B phases

            def phase_A(hb, t):
                """y/vfull/vr/squash (through vsq): aggregation phase."""
                ve = nc.vector if hb == 0 else nc.gpsimd
                vkey = ("dve%d" % hb) if hb == 0 else ("gp%d" % hb)
                dk, ak, gk = "dve%d" % hb, "act%d" % hb, "gp%d" % hb
                pvr_of = {}
                if t == 0:
                    # ---- t0 fast path: c uniform 1/32 -> every o gets the
                    # same y row, so v_raw = (xsum/32) @ W needs no diag
                    # extract: xsum via 1-wide matmuls, then a [h,4]-wide
                    # vfull lands v_raw per pack directly. ----
                    pe_sync("cstb", dk, ak, gk, "dve", "act",
                            *["x%d%d%d" % (hb, pk_, s_)
                              for pk_ in range(NPK) for s_ in range(PK)])
                    ps_y0 = ps2k.tile([P, NPK * PK * 2], F32, tag="p2k")
                    for hc in range(2):
                        for pk in range(NPK):
                            for s in range(PK):
                                for ic in range(4):
                                    MM(
                                        ps_y0[:, hc * 8 + pk * PK + s:
                                              hc * 8 + pk * PK + s + 1],
                                        ap(x_sb[hb, pk], [[1, P]],
                                           off=s * 1024 + ic * H + hc * P),
                                        c0b_sb[:, :1],
                                        start=(ic == 0),
                                        stop=(ic == 3),
                                    )
                    yt0_sb = work.tile([P, NPK * PK * 2], R32, tag="yt0")
                    nc.vector.tensor_copy(yt0_sb[:], ps_y0[:])
                    mark(dk, yt0_sb)
                    pe_sync(dk, "cst")
                    for pk in range(NPK):
                        pvr = ps2k.tile([PK, OF], F32, tag="p2k")
                        for hc in range(2):
                            MM(
                                pvr[:],
                                ap(yt0_sb, [[1, PK]], off=hc * 8 + pk * PK),
                                ap(w_sb, [[1, OF]], off=hc * OF),
                                start=(hc == 0),
                                stop=(hc == 1),
                            )
                        pvr_of[pk] = pvr
                else:
                    # ---- yT[h,(hc,pk,s,o)] = x-stationary @ cT (bf16) ----
                    pe_sync("cstb", dk, ak, gk, "dve", "act",
                            *["x%d%d%d" % (hb, pk_, s_)
                              for pk_ in range(NPK) for s_ in range(PK)])
                    ps_yt = ps2k.tile([P, NPK * H], F32, tag="p2k")
                    for hc in range(2):
                        for pk in range(NPK):
                            for s in range(PK):
                                for ic in range(4):
                                    MM(
                                        ps_yt[:, hc * 256 + pk * P + s * O:
                                              hc * 256 + pk * P + s * O + O],
                                        ap(x_sb[hb, pk], [[1, P]],
                                           off=s * 1024 + ic * H + hc * P),
                                        ap(cT[hb], [[1, O]],
                                           off=ic * 256 + pk * P + s * O),
                                        start=(ic == 0),
                                        stop=(ic == 3),
                                    )
                    yt_sb = work.tile([P, NPK * H], R32, tag="ytsb")
                    nc.scalar.activation(yt_sb[:], ps_yt[:], AF.Copy)
                    mark(ak, yt_sb)

                    # ---- vfull = y @ W (fp32r): [(pk,) 4s*32o', (o,f)] ----
                    pe_sync(ak, "cst")
                    msk_sb = work.tile([P, NPK * OF], R32, tag="bigsb")
                    for pk in range(NPK):
                        pvf = ps2k.tile([P, OF], F32, tag="p2k")
                        for hc in range(2):
                            MM(
                                pvf[:],
                                ap(yt_sb, [[1, P]], off=hc * 256 + pk * P),
                                ap(w_sb, [[1, OF]], off=hc * OF),
                                start=(hc == 0),
                                stop=(hc == 1),
                            )
                        # diag extract mask (release vf bank early per pk)
                        dep(nc.vector.tensor_mul(
                            ap(msk_sb, [[1, OF]], off=pk * OF),
                            pvf[:],
                            ap(mc_sb, [[1, OF]]),
                        ), dcst_a)
                        mark(dk, msk_sb[:, pk * OF:pk * OF + 1])

                # ---- per-sample partition sum (fp32r indicator), per pk ----
                pe_sync(dk)
                vr_sb = work.tile([PK, NPK * OF], F32, tag="vrsb")
                sq_sb = work.tile([PK, NPK * OF], F32, tag="sqsb")
                mag = sm.tile([PK, NPK * O], F32, tag="mag")
                red_ins = []
                for pk in range(NPK):
                    if t == 0:
                        pvr = pvr_of[pk]
                    else:
                        pvr = ps2k.tile([PK, OF], F32, tag="p2k")
                        MM(
                            pvr[:],
                            s4_sb,
                            msk_sb[:, pk * OF:(pk + 1) * OF],
                            start=True, stop=True,
                        )
                    # DVE may read at most one PSUM operand: land vr in
                    # SBUF once, square/scale from there. Copy on Act: the
                    # A-phase Copy block and B-phase Exp block each reload
                    # the act table once per t.
                    nc.scalar.activation(
                        ap(vr_sb, [[1, OF]], off=pk * OF), pvr[:], AF.Copy)
                    mark(ak, vr_sb[:, pk * OF:pk * OF + 1])
                    ve.tensor_mul(
                        ap(sq_sb, [[1, OF]], off=pk * OF),
                        ap(vr_sb, [[1, OF]], off=pk * OF),
                        ap(vr_sb, [[1, OF]], off=pk * OF))
                red_ins.append(nc.vector.reduce_sum(
                    out=mag[:],
                    in_=ap(sq_sb, [[F, NPK * O], [1, F]]),
                    axis=AX.X,
                ).ins)

                # ---- squash: factor = sqrt(mag)/(1+mag), sqrt via rsqrt
                # bit-trick + 2 Newton steps (keeps the Act table on Exp).
                # hb0's scalar chain runs on DVE, hb1's on Pool so the two
                # half-batches don't contend.
                sh_t = sm.tile([PK, NPK * O], I32, tag="sh")
                # int shift is DVE-only (Pool ALU lacks it)
                sh_i = nc.vector.tensor_scalar(
                    out=sh_t[:], in0=aview(mag[:], I32), scalar1=1,
                    scalar2=None,
                    op0=mybir.AluOpType.arith_shift_right)
                for ri_ in red_ins:  # alias reads are not dep-tracked
                    bass._add_dep_helper(sh_i.ins, ri_, sync=True,
                                         reason="aliased mag read")
                r0_t = sm.tile([PK, NPK * O], I32, tag="s0")
                nc.vector.tensor_scalar(
                    out=r0_t[:], in0=sh_t[:], scalar1=-1,
                    scalar2=0x5F3759DF,
                    op0=mybir.AluOpType.mult, op1=mybir.AluOpType.add)
                r = aview(r0_t[:], F32)
                for _ in range(1):  # Newton: r = r*(1.5 - 0.5*m*r^2)
                    a = sm.tile([PK, NPK * O], F32, tag="nta")
                    ve.tensor_mul(a[:], mag[:], r)
                    h = sm.tile([PK, NPK * O], F32, tag="nth")
                    ve.tensor_mul(h[:], a[:], r)
                    w = sm.tile([PK, NPK * O], F32, tag="ntw")
                    ve.tensor_scalar(
                        out=w[:], in0=h[:], scalar1=-0.5, scalar2=1.5,
                        op0=mybir.AluOpType.mult, op1=mybir.AluOpType.add)
                    r2 = sm.tile([PK, NPK * O], F32, tag="ntr")
                    ve.tensor_mul(r2[:], r, w[:])
                    r = r2[:]
                s0 = sm.tile([PK, NPK * O], F32, tag="s0f")
                ve.tensor_mul(s0[:], mag[:], r)  # sqrt = m * rsqrt
                onep = sm.tile([PK, NPK * O], F32, tag="onep")
                ve.tensor_scalar_add(onep[:], mag[:], 1.0)
                rec = sm.tile([PK, NPK * O], F32, tag="rec")
                nc.vector.reciprocal(rec[:], onep[:])
                factor = sm.tile([PK, NPK * O], F32, tag="fac")
                ve.tensor_mul(factor[:], s0[:], rec[:])
                vsq = work.tile([PK, NPK * OF], F32, tag="vsq", bufs=4)
                for pk in range(NPK):
                    ve.tensor_mul(
                        ap(vsq, [[F, O], [1, F]], off=pk * OF),
                        ap(vr_sb, [[F, O], [1, F]], off=pk * OF),
                        ap(factor, [[1, O], [0, F]], off=pk * O),
                    )
                mark(vkey, vsq)
                vstate[hb] = vsq

            def phase_B(hb, t):
                """agreement phase: vT/vp2/zT/bT/softmax (skip on last iter)."""
                vsq = vstate[hb]
                dk, ak, gk = "dve%d" % hb, "act%d" % hb, "gp%d" % hb

                if t == NITER - 1:
                    dq = nc.sync if hb == 0 else nc.scalar
                    dq.dma_start(
                        out=dram_ap(out_d, [[OF, PK], [PK * OF, NPK], [1, OF]],
                                    off=hb * 8 * OF),
                        in_=ap(vsq, [[OF, NPK], [1, OF]]),
                    )
                    return

                # ---- vT chunks: [(o8,f16)%128, (pk, m, s)] ----
                pe_sync(dk, ak, gk)
                ps_vt = ps2k.tile([P, NPK * 4 * PK], F32, tag="p2k")
                for pk in range(NPK):
                    for m in range(4):
                        T(
                            ps_vt[:, (pk * 4 + m) * PK:(pk * 4 + m + 1) * PK],
                            vsq[:, pk * OF + m * P: pk * OF + (m + 1) * P],
                            id_sb[:PK, :PK],
                        )
                vt_sb = work.tile([P, NPK * 4 * PK], F32, tag="vtsb")
                nc.vector.tensor_copy(vt_sb[:], ps_vt[:])

                # ---- VmatT blocks: vp2[p,(m,pk,s,o)] = vtT * blockmask ----
                vp2_sb = work.tile([P, 4 * NPK * PK * O], R32, tag="vp")
                ve2 = nc.gpsimd
                for m in range(4):
                    dep(ve2.tensor_mul(
                        ap(vp2_sb, [[PK * O, NPK], [O, PK], [1, O]],
                           off=m * NPK * PK * O),
                        ap(vt_sb, [[4 * PK, NPK], [1, PK], [0, O]],
                           off=m * PK),
                        ap(bm_sb, [[0, NPK], [0, PK], [1, O]],
                           off=m * O),
                    ), dcst_a)
                mark(gk,
                     vp2_sb[:, 3 * NPK * PK * O:3 * NPK * PK * O + 1])

                # ---- zT = WT-chunk stationary @ VmatT (fp32r, dst 0):
                #      [h%128, (hc, pk, s, o)] ----
                pe_sync(dk, ak, gk)
                ps_zt = ps2k.tile([P, NPK * H], F32, tag="p2k")
                for hc in range(2):
                    for m in range(4):
                        MM(
                            ps_zt[:, hc * NPK * P:(hc + 1) * NPK * P],
                            ap(wt_sb, [[1, P]], off=m * H + hc * P),
                            ap(vp2_sb, [[1, NPK * P]], off=m * NPK * P),
                            start=(m == 0),
                            stop=(m == 3),
                        )
                zt_sb = work.tile([P, NPK * H], BF16, tag="ztsb")
                nc.vector.tensor_copy(zt_sb[:], ps_zt[:])
                mark(dk, zt_sb)

                # ---- bT[i,(ic),(pk,s,o)] = xT-chunk stationary @ zT (bf16),
                #      two 2KB PSUM halves so exp(half0) overlaps half1 ----
                pe_sync(dk, "dve", "act")
                ebt = work.tile([P, 4 * NPK * P], BF16, tag="ebt")
                for half in range(2):
                    ps_bt = ps2k.tile([P, NPK * P * 2], F32, tag="p2k")
                    for ic2 in range(2):
                        ic = half * 2 + ic2
                        for pk in range(NPK):
                            for s in range(PK):
                                for hc in range(2):
                                    MM(
                                        ps_bt[:, ic2 * 256 + pk * P + s * O:
                                              ic2 * 256 + pk * P + s * O + O],
                                        ap(xt_sb[hb, pk], [[1, P]],
                                           off=s * 1024 + hc * I + ic * P),
                                        ap(zt_sb, [[1, O]],
                                           off=hc * NPK * P + pk * P + s * O),
                                        start=(hc == 0),
                                        stop=(hc == 1),
                                    )
                    # ---- softmax exp (b in +-5: exp w/o max-sub) ----
                    nc.scalar.activation(
                        ebt[:, half * 512:(half + 1) * 512],
                        ps_bt[:], AF.Exp)
                mark(ak, ebt)
                ssum = sm.tile([P, 4 * NPK * PK], F32, tag="ssum")
                nc.vector.reduce_sum(
                    out=ssum[:],
                    in_=ap(ebt, [[O, 4 * NPK * PK], [1, O]]),
                    axis=AX.X,
                )
                rsum = sm.tile([P, 4 * NPK * PK], F32, tag="rsum")
                nc.vector.reciprocal(rsum[:], ssum[:])
                cT[hb] = work.tile([P, 4 * NPK * P], BF16, tag="ct%d" % hb,
                                   name="ct_t")
                ve3 = nc.gpsimd
                ve3.tensor_mul(
                    ap(cT[hb], [[O, 4 * NPK * PK], [1, O]]),
                    ap(ebt, [[O, 4 * NPK * PK], [1, O]]),
                    ap(rsum, [[1, 4 * NPK * PK], [0, O]]),
                )
                mark(gk if hb == 1 else dk, cT[hb][:, :1])

            # ---- emission: xT0, A0(t0), xT1, A1(t0), B0, B1, then t=1,2 ----
            build_xt(0)
            phase_A(0, 0)
            build_xt(1)
            phase_A(1, 0)
            phase_B(0, 0)
            phase_B(1, 0)
            for t in range(1, NITER):
                phase_A(0, t)
                phase_A(1, t)
                phase_B(0, t)
                phase_B(1, t)

    if split_waits:
        _split_fat_waits(nc)
    return nc


def _split_fat_waits(nc, maxw=1):
    """Walrus caps sync waits per instruction; split overflow onto extra
    same-engine Drain instructions inserted just before the offender."""
    nsplit = 0
    for blk in nc.m.functions[0].blocks:
        new_insts = []
        for inst in blk.instructions:
            si = getattr(inst, "sync_info", None)
            w = list(si.on_wait) if si is not None and si.on_wait else []
            if len(w) > maxw:
                for k in range(0, len(w) - maxw, maxw):
                    d = mybir.InstDrain(name="I-waitsplit-%d" % nsplit,
                                        ins=[], outs=[])
                    nsplit += 1
                    d.engine = inst.engine
                    d.sync_info = mybir.SyncInfo(on_wait=w[k:k + maxw],
                                                 on_update=[])
                    new_insts.append(d)
                si.on_wait = w[len(w) - maxw:]
            new_insts.append(inst)
        blk.instructions[:] = new_insts
    return nc


_NC_CACHE = None


def make_cst(Wn):
    """fp32 constant blob [128, CSTN] matching the device-side layout."""
    cst = np.zeros((P, CSTN), np.float32)
    # W [h, of] -> [h%128, (hc, of)]
    cst[:, CW:CW + 2 * OF] = Wn.reshape(2, P, OF).transpose(1, 0, 2).reshape(P, 2 * OF)
    # WT [of, h] -> [of%128, (m, h)]
    cst[:, CWT:CWT + 4 * H] = (
        Wn.T.reshape(4, P, H).transpose(1, 0, 2).reshape(P, 4 * H))
    cst[:, CID:CID + P] = np.eye(P, dtype=np.float32)
    for p in range(P):
        o = p % O
        cst[p, CMC + o * F:CMC + (o + 1) * F] = 1.0
    cst[np.arange(P), CS4 + np.arange(P) // O] = 1.0
    for p in range(P):
        for m in range(4):
            cst[p, CBM + m * O + 8 * m + p // F] = 1.0
    return cst


def make_cstb():
    """bf16 constant blob [128, CBN]: identity + uniform 1/32."""
    cb = np.zeros((P, CBN), ml_dtypes.bfloat16)
    cb[:, CBID:CBID + P] = np.eye(P, dtype=ml_dtypes.bfloat16)
    cb[:, CBC0:CBC0 + O] = ml_dtypes.bfloat16(1.0 / O)
    return cb


def make_in_maps(x, W):
    x = np.asarray(x, dtype=np.float32)
    xb = np.ascontiguousarray(x.astype(ml_dtypes.bfloat16))
    Wn = np.ascontiguousarray(np.asarray(W, dtype=np.float32).reshape(H, OF))
    cst = make_cst(Wn)
    cstb = make_cstb()
    xs = xb.reshape(NCORES, S, I, H)
    return [
        {"x": np.ascontiguousarray(xs[c]), "cst": cst, "cstb": cstb}
        for c in range(NCORES)
    ]


def kernel(x: np.ndarray, W: np.ndarray) -> np.ndarray:
    global _NC_CACHE
    if _NC_CACHE is None:
        _NC_CACHE = build_program()
    in_maps = make_in_maps(x, W)
    res = run_bass_kernel_spmd(_NC_CACHE, in_maps, core_ids=list(range(NCORES)))
    out = np.stack([res.results[c]["out"] for c in range(NCORES)])
    return out.reshape(B, O, F)
